# revision 36
# baseline (speedup 1.0000x reference)
"""Single-head causal attention (B=4, S=4096, E=512, D=64) on 8 TRN2 NeuronCores.

Sharding: each batch element is split across a PAIR of cores using a
triangle+rectangle decomposition of the causal score matrix:
  core A (h=0): tri = causal attn of rows 0:2048;  rect = q 2048:3072 x k 0:2048
  core B (h=1): tri = causal attn of rows 2048:4096; rect = q 3072:4096 x k 0:2048
Both cores run the IDENTICAL graph (uniform SPMD); only the input slices
differ. Cores emit unnormalized partials [O^T | sums] which the host
combines (exp uses no max-subtraction, so partials add exactly).

Per-core pipeline (x arrives pre-transposed e-major + pre-cast bf16 from
the host, so the kernel needs no transposes of x at all):
  1. plain chunked DMAs load xT [128, e-chunk, s] into SBUF
  2. packed projections: [WQ|WK]^T @ xT and [WK|WV]^T @ xT (bf16, full array)
     V^T re-transposed to V tiles [sk,64]+ones column
  3. scores: per ki-pair one PSUM tile [128,1024], two row-tiled bf16 matmuls
     (pair of K tiles in partition halves, d=64 contraction each, concurrent)
     one exp over [128,1024] on ACT -> et bf16; causal masks on DVE (bf16)
  4. AV: OT[65,512] += [V|1]^T @ et accumulated over ki in PSUM
  5. DVE copy OT -> SBUF, DMA partials out; host divides/combines.
"""

import math

import numpy as np

_B, _S, _E, _D = 4, 4096, 512, 64
_P = 128
_NC = 8
_TRI = 2048   # tri rows per core
_RQ = 1024    # rect q rows per core
_RK = 2048    # rect k rows per core

_nc_cache = {}


def _lean_drain_and_barrier(self, tick_clock, wait_clock):
    """TileContext exit minus the second all-engine barrier (~4-5us of
    teardown). Bass.__init__ clears the kernel semaphore range in its
    preamble, so a freshly loaded NEFF doesn't depend on exit-time state."""
    import bass_rust

    drain_inst = self.nc.sync.drain()
    wait_clock.add_sem_waits(
        drain_inst.ins,
        bass_rust.ScopedClock({None: tick_clock.global_clock}),
    )
    self.nc.all_engine_barrier()
    popped = self.nc._tile_sem_poison_stack.pop()
    assert popped is self._sem_poison
    # free python-side bookkeeping without emitting clear instructions:
    # Bass.__init__'s preamble clears the kernel sem range on next load.
    sems = list(self.sems.allocated().values())
    if sems:
        sem_nums = [s.num for s in sems]
        self.nc._state.prepend_free_semaphores(sem_nums)
        for poison_set in self.nc._tile_sem_poison_stack:
            poison_set.update(sem_nums)


def _build_nc():
    import concourse.bacc as bacc
    import concourse.mybir as mybir
    import concourse.tile as tile
    from concourse.masks import make_identity

    tile.TileContext._drain_and_barrier = _lean_drain_and_barrier

    f32 = mybir.dt.float32
    f32r = mybir.dt.float32r
    bf16 = mybir.dt.bfloat16
    P = 128
    E, D = _E, _D
    EC = E // P                      # 4 e-chunks
    NT_TRI = _TRI // P               # 16 tri s-tiles
    NT_RQ = _RQ // P                 # 8
    NT_RK = _RK // P                 # 16
    NB_TRI = _TRI // 512             # 4 tri q-blocks
    NB_RQ = _RQ // 512               # 2 rect q-blocks
    scale = 1.0 / math.sqrt(E)

    nc = bacc.Bacc("TRN2", target_bir_lowering=False)
    xa_ext = nc.declare_dram_parameter("xa", [E, _TRI], bf16, isOutput=False)
    xq_ext = nc.declare_dram_parameter("xq", [E, _RQ], bf16, isOutput=False)
    xk_ext = nc.declare_dram_parameter("xk", [E, _RK], bf16, isOutput=False)
    wqk_ext = nc.declare_dram_parameter("wqk", [E, 128], bf16, isOutput=False)
    wkv_ext = nc.declare_dram_parameter("wkv", [E, 128], bf16, isOutput=False)
    mask_ext = nc.declare_dram_parameter("mask", [P, 4, 512], bf16, isOutput=False)
    otri_ext = nc.declare_dram_parameter("otri", [NB_TRI, D + 1, 512], f32, isOutput=True)
    orect_ext = nc.declare_dram_parameter("orect", [NB_RQ, D + 1, 512], f32, isOutput=True)

    with tile.TileContext(nc) as tc:
        with (
            tc.tile_pool(name="const", bufs=1) as const,
            tc.tile_pool(name="big", bufs=1) as big,
            tc.tile_pool(name="work", bufs=4) as work,
            tc.tile_pool(name="expp", bufs=3) as expp,
            tc.tile_pool(name="psx", bufs=2, space="PSUM") as psx,
            tc.tile_pool(name="psa", bufs=2, space="PSUM") as psa,
            tc.tile_pool(name="pso", bufs=2, space="PSUM") as pso,
        ):
            ident_bf = const.tile([P, P], bf16)
            make_identity(nc, ident_bf)

            wqk_sb = const.tile([P, EC, 128], bf16, name="wqk")
            wkv_sb = const.tile([P, EC, 128], bf16, name="wkv")
            mask_sb = const.tile([P, 4, 512], bf16, name="mask")

            # transposed activations, bf16 [e_local, e_chunk, s]
            xaT = big.tile([P, EC, _TRI], bf16, name="xaT")
            xqT = big.tile([P, EC, _RQ], bf16, name="xqT")
            xkT = big.tile([P, EC, _RK], bf16, name="xkT")
            # projections (bf16): q/k in [d, s] layout; v as [sk, tile, D+1]
            qT_tri = big.tile([P, _TRI], bf16, name="qTtri")
            kT_tri = big.tile([P, NT_TRI // 2, P], bf16, name="kTtri")
            qT_rq = big.tile([P, _RQ], bf16, name="qTrq")
            kT_rk = big.tile([P, NT_RK // 2, P], bf16, name="kTrk")
            vT_tri = big.tile([64, _TRI], bf16, name="vTtri")
            vT_rk = big.tile([64, _RK], bf16, name="vTrk")
            vp_tri = big.tile([P, NT_TRI, D + 1], bf16, name="vptri")
            vp_rk = big.tile([P, NT_RK, D + 1], bf16, name="vprk")
            nc.vector.memset(vp_tri[:, :, D : D + 1], 1.0)
            nc.vector.memset(vp_rk[:, :, D : D + 1], 1.0)

            def load_xt(x_ext, xT, slices):
                """x arrives pre-transposed (e-major) from the host; plain DMA
                per s-slice into the [128, EC, s] SBUF layout."""
                for s0, s1 in slices:
                    nc.sync.dma_start(
                        xT[:, :, s0:s1],
                        x_ext[:, s0:s1].rearrange("(c p) s -> p c s", p=P),
                    )

            def copy_q_dup(qT, j, rows):
                # duplicate q rows into both partition halves for row-tiled MMs
                # (partition-shifting copies must stay on DVE; gpsimd is
                # partition-local and silently corrupts them)
                nc.vector.tensor_copy(out=qT[0:64, j * 512 : (j + 1) * 512], in_=rows)
                nc.vector.tensor_copy(out=qT[64:128, j * 512 : (j + 1) * 512], in_=rows)

            def copy_k_pairs(kT, j, rows):
                # chunk j holds ki tiles 4j..4j+3 -> pairs 2j, 2j+1
                # view rows [64, 512] as [64, pair(2), half(2), 128]
                v = rows.rearrange("p (t u q) -> p t u q", t=2, u=2)
                for half in range(2):
                    nc.vector.tensor_copy(
                        out=kT[half * 64 : half * 64 + 64, 2 * j : 2 * j + 2, :],
                        in_=v[:, :, half, :],
                    )

            def consume_qk(qT, kT):
                # wqk = [WQ|WK]: psum rows 0:64 = Q^T, rows 64:128 = K^T
                def f(j, pq):
                    copy_q_dup(qT, j, pq[0:64, :])
                    copy_k_pairs(kT, j, pq[64:128, :])
                return f

            def consume_kv(kT, vT):
                # wkv = [WK|WV]: psum rows 0:64 = K^T, rows 64:128 = V^T
                def f(j, pq):
                    copy_k_pairs(kT, j, pq[0:64, :])
                    nc.vector.tensor_copy(out=vT[:, j * 512 : (j + 1) * 512], in_=pq[64:128, :])
                return f

            def consume_q(qT):
                def f(j, pq):
                    copy_q_dup(qT, j, pq[0:64, :])
                return f

            def consume_v(vT):
                def f(j, pq):
                    nc.vector.tensor_copy(out=vT[:, j * 512 : (j + 1) * 512], in_=pq[64:128, :])
                return f

            def v_retranspose(vT, vp, tiles):
                """vT [64, s] bf16 -> vp [sk(P), tile, 64] via PE transpose."""
                for k in tiles:
                    pv = psx.tile([P, 64], bf16, tag="x", name="pvt")
                    nc.tensor.matmul(
                        pv,
                        vT[:, k * P : (k + 1) * P],
                        ident_bf[0:64, 0:64],
                        is_transpose=True,
                    )
                    nc.vector.tensor_copy(out=vp[:, k, 0:D], in_=pv[:, 0:64])

            def attention_pairs(qT, kT, vp, j, prs, npr, masked, po,
                                start_pr=0, stop_pr=None):
                """emit score/exp/AV for ki pairs `prs` of q-block j.
                start_pr/stop_pr bound this po's accumulation group."""
                if stop_pr is None:
                    stop_pr = npr - 1
                for pr in prs:
                    ps = psa.tile([P, 1024], f32, tag="a", name="psc")
                    for half in range(2):
                        nc.tensor.matmul(
                            ps[:, half * 512 : (half + 1) * 512],
                            kT[half * 64 : half * 64 + 64, pr, :],
                            qT[half * 64 : half * 64 + 64, j * 512 : (j + 1) * 512],
                            start=True,
                            stop=True,
                            skip_group_check=True,
                        )
                    et = expp.tile([P, 1024], bf16, name="et")
                    nc.scalar.activation(
                        et, ps, mybir.ActivationFunctionType.Exp, scale=scale
                    )
                    if masked and pr >= npr - 2:
                        r0 = 2 * (pr - (npr - 2))
                        for half in range(2):
                            nc.vector.tensor_tensor(
                                et[:, half * 512 : (half + 1) * 512],
                                et[:, half * 512 : (half + 1) * 512],
                                mask_sb[:, r0 + half, :],
                                mybir.AluOpType.mult,
                            )
                    for half in range(2):
                        ki = 2 * pr + half
                        nc.tensor.matmul(
                            po,
                            vp[:, ki, :],
                            et[:, half * 512 : (half + 1) * 512],
                            start=(pr == start_pr and half == 0),
                            stop=(pr == stop_pr and half == 1),
                            skip_group_check=True,
                        )

            def block_out(po, out_ext, j):
                ot = work.tile([D + 1, 512], f32, name="otc")
                nc.vector.tensor_copy(out=ot, in_=po)
                nc.sync.dma_start(out_ext[j], ot)

            def attention_block(qT, kT, vp, j, nk, masked, out_ext):
                """one 512-wide q-block: accumulate OT[65,512] over nk ki tiles."""
                npr = nk // 2
                po = pso.tile([D + 1, 512], f32, tag="o", name="po")
                attention_pairs(qT, kT, vp, j, range(npr), npr, masked, po)
                block_out(po, out_ext, j)

            def proj_chunk(xT, j, dst_psums):
                for w_sb, consume in dst_psums:
                    pq = psx.tile([P, 512], f32, tag="x", name="pproj")
                    for c in range(EC):
                        nc.tensor.matmul(
                            pq,
                            w_sb[:, c, :],
                            xT[:, c, j * 512 : (j + 1) * 512],
                            start=(c == 0),
                            stop=(c == EC - 1),
                        )
                    consume(j, pq)

            # ---- emission order drives scheduler priorities and the
            # sync-queue (DMA dispatch) order ----
            load_xt(xa_ext, xaT, [(0, 256)])
            nc.sync.dma_start(wqk_sb, wqk_ext.rearrange("(c p) d -> p c d", p=P))
            nc.sync.dma_start(wkv_sb, wkv_ext.rearrange("(c p) d -> p c d", p=P))
            load_xt(xa_ext, xaT, [(256, 512)]
                    + [(j * 512, (j + 1) * 512) for j in range(1, 4)])
            nc.sync.dma_start(mask_sb, mask_ext[:, :, :])
            load_xt(xq_ext, xqT, [(0, _RQ)])
            load_xt(xk_ext, xkT, [(j * 512, (j + 1) * 512) for j in range(4)])
            tri_packs = [
                (wqk_sb, consume_qk(qT_tri, kT_tri)),
                (wkv_sb, consume_v(vT_tri)),
            ]
            def tri_prep(j):
                proj_chunk(xaT, j, tri_packs)
                v_retranspose(vT_tri, vp_tri, range(4 * j, 4 * j + 4))
            # prefetch distance 2: block j's attention is emitted before
            # prep j+2, so ready attention work always outranks prep and
            # the next block's inputs are computed during this block.
            tri_prep(0)
            tri_prep(1)
            def rk_prep(c4):
                proj_chunk(xkT, c4, [(wkv_sb, consume_kv(kT_rk, vT_rk))])
                v_retranspose(vT_rk, vp_rk, range(4 * c4, 4 * c4 + 4))

            for j in range(NB_TRI):
                attention_block(
                    qT_tri, kT_tri, vp_tri, j, 4 * j + 4, True, otri_ext
                )
                if j + 2 < NB_TRI:
                    tri_prep(j + 2)
                if j == 0:
                    # rect q projection is small; get it out of the way early
                    for jq in range(NB_RQ):
                        proj_chunk(xqT, jq, [(wqk_sb, consume_q(qT_rq))])
                elif j <= 2:
                    # hoist early rk chunks so rect scores are ready the
                    # moment tri attention drains
                    rk_prep(j - 1)
            # rect attention streams per rk chunk as its prep lands, with
            # both rect q-blocks' OT accumulations live simultaneously.
            po_rect = [
                pso.tile([D + 1, 512], f32, tag="o", name=f"por{jb}")
                for jb in range(NB_RQ)
            ]
            NPR_R = NT_RK // 2
            for c4 in range(NT_RK // 4):
                if c4 >= 2:
                    rk_prep(c4)
                for jb in range(NB_RQ):
                    attention_pairs(
                        qT_rq, kT_rk, vp_rk, jb,
                        range(2 * c4, 2 * c4 + 2), NPR_R, False, po_rect[jb],
                    )
            for jb in range(NB_RQ):
                block_out(po_rect[jb], orect_ext, jb)

    nc.finalize()
    return nc


def _get_nc():
    if "nc" not in _nc_cache:
        _nc_cache["nc"] = _build_nc()
    return _nc_cache["nc"]


def _masks_bf16():
    import ml_dtypes

    k = np.arange(4)[:, None, None]
    p = np.arange(_P)[None, :, None]
    f = np.arange(512)[None, None, :]
    m = (f >= k * _P + p).astype(ml_dtypes.bfloat16)  # [4, 128, 512]
    return np.ascontiguousarray(m.transpose(1, 0, 2))  # [128, 4, 512]


def _in_maps(x, WQ, WK, WV):
    import ml_dtypes

    bf = ml_dtypes.bfloat16
    wqk = np.ascontiguousarray(np.concatenate([WQ, WK], axis=1)).astype(bf)
    wkv = np.ascontiguousarray(np.concatenate([WK, WV], axis=1)).astype(bf)
    masks = _masks_bf16()
    # pre-cast + pre-transpose once: kernel takes e-major bf16 slices
    xbT = np.ascontiguousarray(x.transpose(0, 2, 1).astype(bf))  # [B, E, S]
    maps = []
    for c in range(_NC):
        b, h = c // 2, c % 2
        if h == 0:
            xa = xbT[b, :, 0:2048]
            xq = xbT[b, :, 2048:3072]
        else:
            xa = xbT[b, :, 2048:4096]
            xq = xbT[b, :, 3072:4096]
        xk = xbT[b, :, 0:2048]
        maps.append(
            {
                "xa": np.ascontiguousarray(xa),
                "xq": np.ascontiguousarray(xq),
                "xk": np.ascontiguousarray(xk),
                "wqk": wqk,
                "wkv": wkv,
                "mask": masks,
            }
        )
    return maps


def _blocks_to_sq(o):
    """[nb, 65, 512] -> (O [nb*512, 64], s [nb*512])"""
    O = np.ascontiguousarray(o[:, :_D, :]).transpose(0, 2, 1).reshape(-1, _D)
    s = np.ascontiguousarray(o[:, _D, :]).reshape(-1)
    return O.astype(np.float64), s.astype(np.float64)


def _combine(outs):
    out = np.empty((_B, _S, _D), dtype=np.float32)
    for b in range(_B):
        ra, rb = outs[2 * b], outs[2 * b + 1]
        Oa, sa = _blocks_to_sq(np.asarray(ra["otri"]))
        out[b, 0:2048] = (Oa / sa[:, None]).astype(np.float32)
        Ob, sb = _blocks_to_sq(np.asarray(rb["otri"]))
        Oar, sar = _blocks_to_sq(np.asarray(ra["orect"]))
        Obr, sbr = _blocks_to_sq(np.asarray(rb["orect"]))
        Ob[0:1024] += Oar
        sb[0:1024] += sar
        Ob[1024:2048] += Obr
        sb[1024:2048] += sbr
        out[b, 2048:4096] = (Ob / sb[:, None]).astype(np.float32)
    return out


def kernel(x, WQ, WK, WV):
    x = np.asarray(x, dtype=np.float32)
    WQ = np.asarray(WQ, dtype=np.float32)
    WK = np.asarray(WK, dtype=np.float32)
    WV = np.asarray(WV, dtype=np.float32)
    from concourse.bass_utils import run_bass_kernel_spmd

    nc = _get_nc()
    res = run_bass_kernel_spmd(nc, _in_maps(x, WQ, WK, WV), core_ids=list(range(_NC)))
    return _combine(res.results)
